# revision 26
# baseline (speedup 1.0000x reference)
"""CharRNN Trainium2 kernel (8-core data-parallel), bf16 scan, 2 ACT chains.

Math: h_t = tanh(emb[x_t] @ Wx + h_{t-1} @ Wh + b_rnn); logits = (h_T * mask) @ Wd + bd.

Key transformations:
 1. emb[x] @ Wx == (emb @ Wx)[x]: embedding + input projection fold into a tiny
    table M = emb @ Wx + b_rnn [256, 10]; the host gathers U = M[x] per batch
    shard (indexing only) and ships it in on-chip layout as bf16 (halves DMA,
    enables 1-cycle/row PE matmuls vs fp32's 4).
 2. The serial wall is the per-step PE->ACT->PE round trip, measured on HW as
    sem(52) + MM(231) + sem(38) + ACT tanh(331) = 652ns/step; the two ACT ops
    per step also sum to ~654ns, so the scan sits exactly at both the latency
    and the ACT-throughput limit. The batch lanes are split into 2 phase-
    shifted chains (86/85 lanes). Variants measured SLOWER on this walrus
    build: all-DVE fused tanh5(psum+u) chains (830ns/step — strict-FIFO
    phase coupling), lane-split ACT+DVE activation of one chain's tile
    (tile-granular WAW tracking serializes the writers), and a third
    independent DVE-poly chain (PE queue saturates on 3 chains' matmuls +
    identity injections).
 3. U enters each chain's PSUM bank via an identity matmul covering 2 steps
    (start=True); the Wh matmuls then accumulate on top (start=False),
    keeping the per-step matmul stop-only and the U add off the critical
    path. Startup: pair 0 rides in the ew DMA; pairs 1-3 arrive via a second
    DMA on the Scalar queue whose descriptor generation runs concurrently,
    giving 8 steps of headroom before the first Sync-queue chunk must land
    (removes the early-scan chunk stalls).
 4. Tail: dense head in bf16 (fp32 stationaries forced 4-pass LOW_HIGH
    matmuls) with bd folded in as a 121st contract row against a constant-1
    row of the masked-h operand; both halves' logits leave in ONE DMA.
    End-to-end max rel err 8.2e-3 vs the 2e-2 gate (bf16 carry dominates).

Device layout (per core, batch shard 2048 padded to 2052 = 12 groups x 171):
  partitions 10g+h (g in [0,12), h in [0,10)) hold hidden unit h of batch
  group g; the free dim holds that group's 171 batch lanes, split into chains
  at [0,86,171]. The RNN matmul uses a block-diagonal Wh [120,120] bf16.
  Each chain has its own PSUM pool (tile-granular dependency tracking would
  otherwise serialize chains sharing a tile). h is carried in bf16; the last
  step's activations stay f32. Only PE writes PSUM.
"""
import numpy as np
import ml_dtypes

import concourse.bass as bass
import concourse.mybir as mybir
from concourse.tile import TileContext
from concourse.bass_utils import run_bass_kernel_spmd

# problem shape (hardcoded per contract)
B, T, V, E, H, L = 16384, 100, 256, 50, 10, 15
N_CORES = 8
BC = B // N_CORES          # 2048 batch per core
G = 12                     # partition groups
BG = 171                   # batch lanes per group
CB = [0, 86, 171]  # chain lane bounds (both chains on ACT)
NCH = 2
BP = G * BG                # 2052 padded batch per core
NF = T * BG                # u free dim = 17100
EWP = 4                    # leading u pairs excluded from the chunk DMAs:
                           # pair 0 rides in ew; pairs 1-3 go in a parallel
                           # DMA on the Scalar queue (concurrent DGE gen)
CHUNK_STEPS = [12, 16, 16, 16, 16, 16]  # steps 8..100
assert sum(CHUNK_STEPS) == T - 2 * EWP

F32 = mybir.dt.float32
BF16 = mybir.dt.bfloat16
NP_BF16 = ml_dtypes.bfloat16

# degree-5 odd minimax fit of tanh on [-0.62, 0.62] (max err 2.1e-5)
TC0, TC1, TC2 = 0.9997536862008579, -0.3279690798565145, 0.10333010061243125

_TANH5 = None


def _register_tanh5():
    """Register the TANH5_ANT custom DVE op (idempotent). The per-NEFF DVE
    table generator resolves ops by name from dve_ops.OPS, so registration
    must precede compile; the sha pin is computed from the lowered uops."""
    global _TANH5
    if _TANH5 is not None:
        return _TANH5
    import concourse.dve_ops as dve_ops
    from concourse.dve_spec import Spec, Src0, C0, C1, C2, sq, lower
    from concourse.dve_uop import DveOpSpec

    for op in dve_ops.OPS:
        if op.name == "TANH5_ANT":
            _TANH5 = op
            return op
    t = sq(Src0)
    spec = Spec(body=Src0 * (C0 + t * (C1 + t * C2)))
    shas = {}
    for ver in ("v3", "v4"):
        uops = lower(spec, ver=ver)
        shas[ver] = DveOpSpec(
            name="TANH5_ANT", opcode=0, uops=uops, rd1_en=False
        ).sha(ver)
    op = dve_ops.DveOp("TANH5_ANT", spec, subdim=False, uops_sha=shas)
    dve_ops.OPS.append(op)
    dve_ops.CUSTOM_DVE_SPECS[op.name] = spec
    dve_ops._SUB_OPCODE_FOR_NAME[op.name] = (
        dve_ops._CUSTOM_DVE_ROW_BASE + len(dve_ops.OPS) - 1
    )
    _TANH5 = op
    return op


def _split_multi_waits(nc):
    """This walrus build rejects >1 sem wait per instruction; hoist extras
    onto NoOps just before, on the same (in-order) engine queue."""
    uid = 0
    for f in nc.m.functions:
        for bb in f.blocks:
            if not any(
                i.sync_info is not None and len(i.sync_info.on_wait) > 1
                for i in bb.instructions
            ):
                continue
            new_list = []
            for inst in bb.instructions:
                si = inst.sync_info
                if si is not None and len(si.on_wait) > 1:
                    waits = list(si.on_wait)
                    for w in waits[:-1]:
                        uid += 1
                        new_list.append(
                            mybir.InstNoOp(
                                name=f"WS-{uid}",
                                engine=inst.engine,
                                bass_nofuse=True,
                                sync_info=mybir.SyncInfo(on_wait=[w], on_update=[]),
                            )
                        )
                    inst.sync_info = mybir.SyncInfo(
                        on_wait=[waits[-1]], on_update=list(si.on_update)
                    )
                new_list.append(inst)
            bb.instructions = new_list


_NC_CACHE = None


def _build_nc():
    global _NC_CACHE
    if _NC_CACHE is not None:
        return _NC_CACHE
    nc = bass.Bass(trn_type="TRN2")
    # eye | wh | u-pair-0 packed bf16 so one DMA unblocks the whole scan start
    ew_d = nc.dram_tensor("ew", [G * H, 2 * G * H + 2 * BG], BF16, kind="ExternalInput")
    # pairs 1-3, DMA'd concurrently from the Scalar queue: 8 steps of headroom
    # before the first Sync-queue chunk DMA must land
    u03_d = nc.dram_tensor("u03", [G * H, 2 * (EWP - 1) * BG], BF16, kind="ExternalInput")
    u_d = nc.dram_tensor("u", [G * H, NF - 2 * EWP * BG], BF16, kind="ExternalInput")
    # [wd half0 (90) | wd half1 (90) | mask (171)] bf16 over 121 partitions;
    # row 120 = [bd tile | bd tile | ones] (bias contract row / hm ones row)
    wdb_d = nc.dram_tensor("wdb", [G * H + 1, 351], BF16, kind="ExternalInput")
    o_d = nc.dram_tensor("o", [90, 2 * BG], F32, kind="ExternalOutput")

    with TileContext(nc) as tc:
        with (
            tc.tile_pool(name="const", bufs=1) as cpool,
            tc.tile_pool(name="u", bufs=1) as upool,
            tc.tile_pool(name="work", bufs=4) as wpool,
            tc.tile_pool(name="fin", bufs=1) as fpool,
            tc.tile_pool(name="ps0", bufs=3, space="PSUM") as pp0,
            tc.tile_pool(name="ps1", bufs=3, space="PSUM") as pp1,
        ):
            ppools = [pp0, pp1]
            # the scan-unblocking DMA goes on the GpSimd queue: its preamble
            # retires first and its DMA issue cost is ~25ns vs Sync's ~565ns
            t_ew = cpool.tile([G * H, 2 * G * H + 2 * BG], BF16, tag="ew")
            nc.gpsimd.dma_start(out=t_ew[:], in_=ew_d[:])
            t_eye = t_ew[:, 0:G * H]
            t_wh = t_ew[:, G * H:2 * G * H]
            t_u03 = cpool.tile([G * H, 2 * (EWP - 1) * BG], BF16, tag="u03")
            nc.scalar.dma_start(out=t_u03[:], in_=u03_d[:])

            # u chunk tiles; pair p (2 steps) occupies cols [342p, 342p+342)
            # grouped per chain: [c0(s),c0(s+1) | c1(s),c1(s+1) | ...]
            uts = []
            step0 = 2 * EWP
            for k, ns in enumerate(CHUNK_STEPS):
                ut = upool.tile([G * H, ns * BG], BF16, tag=f"u{k}")
                nc.sync.dma_start(
                    out=ut[:],
                    in_=u_d[:, (step0 - 2 * EWP) * BG:(step0 - 2 * EWP + ns) * BG],
                )
                uts.append((ut, step0))
                step0 += ns
            # pair p -> (tile, base column of its 342-col chain-grouped block)
            pair_src = {0: (t_ew, 2 * G * H)}
            for p in range(1, EWP):
                pair_src[p] = (t_u03, 2 * BG * (p - 1))
            for (ut, s0), ns in zip(uts, CHUNK_STEPS):
                for p in range(s0 // 2, (s0 + ns) // 2):
                    pair_src[p] = (ut, (p - s0 // 2) * 2 * BG)

            # tail constants (dense head) arrive long before they're needed
            t_wdb = cpool.tile([G * H + 1, 351], BF16, tag="wdb")
            nc.sync.dma_start(out=t_wdb[:], in_=wdb_d[:])
            t_mask = t_wdb[0:G * H, 180:351]

            # masked-h operand for the head: rows 0..119 = h_T * mask, row 120
            # = 1.0 (contracts against the bd row of the stationary). Engines
            # can't start a write at partition 120, so set the whole tile to
            # 1.0 and let the mask-muls overwrite rows 0..119.
            hm = fpool.tile([G * H + 1, BG], BF16, tag="hm")
            nc.vector.memset(hm[:], 1.0)

            hs = []
            for c in range(NCH):
                hc = wpool.tile([G * H, CB[c + 1] - CB[c]], BF16, tag=f"h{c}")
                nc.vector.memset(hc[:], 0.0)
                hs.append(hc)

            for p in range(T // 2):
                ut, pbase = pair_src[p]
                banks = []
                for c in range(NCH):
                    w = CB[c + 1] - CB[c]
                    ps = ppools[c].tile([G * H, 2 * w], F32, tag=f"ps{c}")
                    off = pbase + 2 * CB[c]
                    nc.tensor.matmul(
                        ps[:], t_eye, ut[:, off:off + 2 * w],
                        start=True, stop=False,
                    )
                    banks.append(ps)
                for s in range(2):
                    step = 2 * p + s
                    last = step == T - 1
                    for c in range(NCH):
                        w = CB[c + 1] - CB[c]
                        sl = banks[c][:, s * w:(s + 1) * w]
                        nc.tensor.matmul(
                            sl, t_wh, hs[c][:],
                            start=False, stop=True, skip_group_check=True,
                        )
                        pool = fpool if last else wpool
                        h_new = pool.tile(
                            [G * H, w],
                            F32 if last else BF16,
                            tag=(f"fh{c}" if last else f"h{c}"),
                        )
                        nc.scalar.activation(
                            h_new[:], sl, mybir.ActivationFunctionType.Tanh
                        )
                        hs[c] = h_new

            for c in range(NCH):
                nc.vector.tensor_mul(
                    hm[0:G * H, CB[c]:CB[c + 1]], hs[c][:],
                    t_mask[:, CB[c]:CB[c + 1]],
                )
            ob = fpool.tile([90, 2 * BG], F32, tag="ob")
            for half in range(2):
                # reuse the chain tag: a new tag would cost another bufs x bank
                po = ppools[half].tile([90, BG], F32, tag=f"ps{half}")
                nc.tensor.matmul(
                    po[:], t_wdb[:, 90 * half:90 * (half + 1)], hm[:],
                    start=True, stop=True,
                )
                nc.vector.tensor_copy(ob[:, BG * half:BG * (half + 1)], po[:])
            nc.gpsimd.dma_start(out=o_d[:], in_=ob[:])

    _split_multi_waits(nc)
    _NC_CACHE = nc
    return nc


def _prepare_in_maps(x, emb, Wx, Wh, b_rnn, Wd, bd, drop_mask):
    x = np.asarray(x)
    emb = np.asarray(emb, dtype=np.float32)
    Wx = np.asarray(Wx, dtype=np.float32)
    Wh = np.asarray(Wh, dtype=np.float32)
    b_rnn = np.asarray(b_rnn, dtype=np.float32)
    Wd = np.asarray(Wd, dtype=np.float32)
    bd = np.asarray(bd, dtype=np.float32)
    drop_mask = np.asarray(drop_mask, dtype=np.float32)

    M = emb @ Wx + b_rnn  # [V, H] fused embedding+input-proj table
    Mb = M.astype(NP_BF16)

    wh_blk = np.zeros((G * H, G * H), np.float32)
    wd_blk = np.zeros((G * H, 180), np.float32)
    for a in range(G):
        wh_blk[10 * a:10 * a + 10, 10 * a:10 * a + 10] = Wh
        half, b6 = divmod(a, 6)
        wd_blk[10 * a:10 * a + 10, 90 * half + 15 * b6:90 * half + 15 * b6 + 15] = Wd
    ew_base = np.concatenate(
        [np.eye(G * H, dtype=np.float32), wh_blk], axis=1
    ).astype(NP_BF16)

    in_maps = []
    for c in range(N_CORES):
        xs = x[c * BC:(c + 1) * BC].astype(np.int64)
        u = np.zeros((BP, T, H), NP_BF16)
        u[:BC] = Mb[xs]
        # [120, 17100]: u_dev[10g+h, 171t+j] = u[171g+j, t, h]
        u_dev = (
            u.reshape(G, BG, T, H).transpose(0, 3, 2, 1).reshape(G * H, NF)
        )
        # regroup columns per 2-step pair into chain blocks:
        # pair p -> [ch0(s), ch0(s+1), ch1(s), ch1(s+1), ...]
        v = u_dev.reshape(G * H, T // 2, 2, BG)
        u_dev = np.ascontiguousarray(
            np.concatenate(
                [
                    v[:, :, :, CB[ci]:CB[ci + 1]].reshape(
                        G * H, T // 2, 2 * (CB[ci + 1] - CB[ci])
                    )
                    for ci in range(NCH)
                ],
                axis=2,
            ).reshape(G * H, NF)
        )
        mp = np.zeros((BP, H), np.float32)
        mp[:BC] = drop_mask[c * BC:(c + 1) * BC]
        mask_dev = mp.reshape(G, BG, H).transpose(0, 2, 1).reshape(G * H, BG)
        wdb = np.zeros((G * H + 1, 351), np.float32)
        wdb[0:G * H, 0:180] = wd_blk
        wdb[0:G * H, 180:351] = mask_dev
        wdb[G * H, 0:90] = np.tile(bd, 6)
        wdb[G * H, 90:180] = np.tile(bd, 6)
        wdb[G * H, 180:351] = 1.0
        ew = np.ascontiguousarray(
            np.concatenate([ew_base, u_dev[:, 0:2 * BG]], axis=1)
        )
        u03 = np.ascontiguousarray(u_dev[:, 2 * BG:2 * EWP * BG])
        u_rest = np.ascontiguousarray(u_dev[:, 2 * EWP * BG:])
        in_maps.append(
            {"ew": ew, "u03": u03, "u": u_rest, "wdb": wdb.astype(NP_BF16)}
        )
    return in_maps


def _assemble(results):
    logits = np.empty((B, L), np.float32)
    for c in range(N_CORES):
        o = results[c]["o"]  # [90, 342]
        parts = []
        for half in range(2):
            oh = o[:, BG * half:BG * (half + 1)]  # [90, 171]
            parts.append(oh.reshape(6, 15, BG).transpose(0, 2, 1).reshape(6 * BG, 15))
        full = np.concatenate(parts, axis=0)  # [2052, 15]
        logits[c * BC:(c + 1) * BC] = full[:BC]
    return logits


_LAST_RES = None


def kernel(x, emb, Wx, Wh, b_rnn, Wd, bd, drop_mask, _trace=False):
    global _LAST_RES
    nc = _build_nc()
    in_maps = _prepare_in_maps(x, emb, Wx, Wh, b_rnn, Wd, bd, drop_mask)
    res = run_bass_kernel_spmd(
        nc, in_maps, core_ids=list(range(N_CORES)), trace=_trace
    )
    _LAST_RES = res
    out = _assemble(res.results)
    if _trace:
        kernel.last_exec_time_ns = res.exec_time_ns
    return out



# revision 29
# speedup vs baseline: 1.0400x; 1.0400x over previous
"""CharRNN Trainium2 kernel (8-core data-parallel), bf16 scan, 2 ACT chains.

Math: h_t = tanh(emb[x_t] @ Wx + h_{t-1} @ Wh + b_rnn); logits = (h_T * mask) @ Wd + bd.

Key transformations:
 1. emb[x] @ Wx == (emb @ Wx)[x]: embedding + input projection fold into a tiny
    table M = emb @ Wx + b_rnn [256, 10]; the host gathers U = M[x] per batch
    shard (indexing only) and ships it in on-chip layout as bf16 (halves DMA,
    enables 1-cycle/row PE matmuls vs fp32's 4).
 2. The serial wall is the per-step PE->ACT->PE round trip, measured on HW as
    sem(52) + MM(231) + sem(38) + ACT tanh(331) = 652ns/step; the two ACT ops
    per step also sum to ~654ns, so the scan sits exactly at both the latency
    and the ACT-throughput limit. The batch lanes are split into 2 phase-
    shifted chains (86/85 lanes). Variants measured SLOWER on this walrus
    build: all-DVE fused tanh5(psum+u) chains (830ns/step — strict-FIFO
    phase coupling), lane-split ACT+DVE activation of one chain's tile
    (tile-granular WAW tracking serializes the writers), and a third
    independent DVE-poly chain (PE queue saturates on 3 chains' matmuls +
    identity injections).
 3. U enters each chain's PSUM bank via an identity matmul covering 2 steps
    (start=True); the Wh matmuls then accumulate on top (start=False),
    keeping the per-step matmul stop-only and the U add off the critical
    path. Startup: pair 0 rides in the ew DMA; pairs 1-3 arrive via a second
    DMA on the Scalar queue whose descriptor generation runs concurrently,
    giving 8 steps of headroom before the first Sync-queue chunk must land
    (removes the early-scan chunk stalls).
 4. Tail: dense head in bf16 (fp32 stationaries forced 4-pass LOW_HIGH
    matmuls) with bd folded in as a 121st contract row against a constant-1
    row of the masked-h operand; both halves' logits leave in ONE DMA.
    End-to-end max rel err 8.2e-3 vs the 2e-2 gate (bf16 carry dominates).

Device layout (per core, batch shard 2048 padded to 2052 = 12 groups x 171):
  partitions 10g+h (g in [0,12), h in [0,10)) hold hidden unit h of batch
  group g; the free dim holds that group's 171 batch lanes, split into chains
  at [0,86,171]. The RNN matmul uses a block-diagonal Wh [120,120] bf16.
  Each chain has its own PSUM pool (tile-granular dependency tracking would
  otherwise serialize chains sharing a tile). h is carried in bf16; the last
  step's activations stay f32. Only PE writes PSUM.
"""
import numpy as np
import ml_dtypes

import concourse.bass as bass
import concourse.mybir as mybir
from concourse.tile import TileContext
from concourse.bass_utils import run_bass_kernel_spmd

# problem shape (hardcoded per contract)
B, T, V, E, H, L = 16384, 100, 256, 50, 10, 15
N_CORES = 8
BC = B // N_CORES          # 2048 batch per core
G = 12                     # partition groups
BG = 171                   # batch lanes per group
CB = [0, 86, 171]  # chain lane bounds (both chains on ACT)
NCH = 2
BP = G * BG                # 2052 padded batch per core
NF = T * BG                # u free dim = 17100
EWP = 4                    # leading u pairs excluded from the chunk DMAs:
                           # pair 0 rides in ew; pairs 1-3 go in a parallel
                           # DMA on the Scalar queue (concurrent DGE gen)
CHUNK_STEPS = [12, 16, 16, 16, 16, 16]  # steps 8..100
assert sum(CHUNK_STEPS) == T - 2 * EWP

F32 = mybir.dt.float32
BF16 = mybir.dt.bfloat16
NP_BF16 = ml_dtypes.bfloat16

# degree-5 odd minimax fit of tanh on [-0.62, 0.62] (max err 2.1e-5)
TC0, TC1, TC2 = 0.9997536862008579, -0.3279690798565145, 0.10333010061243125

_TANH5 = None


def _register_tanh5():
    """Register the TANH5_ANT custom DVE op (idempotent). The per-NEFF DVE
    table generator resolves ops by name from dve_ops.OPS, so registration
    must precede compile; the sha pin is computed from the lowered uops."""
    global _TANH5
    if _TANH5 is not None:
        return _TANH5
    import concourse.dve_ops as dve_ops
    from concourse.dve_spec import Spec, Src0, C0, C1, C2, sq, lower
    from concourse.dve_uop import DveOpSpec

    for op in dve_ops.OPS:
        if op.name == "TANH5_ANT":
            _TANH5 = op
            return op
    t = sq(Src0)
    spec = Spec(body=Src0 * (C0 + t * (C1 + t * C2)))
    shas = {}
    for ver in ("v3", "v4"):
        uops = lower(spec, ver=ver)
        shas[ver] = DveOpSpec(
            name="TANH5_ANT", opcode=0, uops=uops, rd1_en=False
        ).sha(ver)
    op = dve_ops.DveOp("TANH5_ANT", spec, subdim=False, uops_sha=shas)
    dve_ops.OPS.append(op)
    dve_ops.CUSTOM_DVE_SPECS[op.name] = spec
    dve_ops._SUB_OPCODE_FOR_NAME[op.name] = (
        dve_ops._CUSTOM_DVE_ROW_BASE + len(dve_ops.OPS) - 1
    )
    _TANH5 = op
    return op


def _split_multi_waits(nc):
    """This walrus build rejects >1 sem wait per instruction; hoist extras
    onto NoOps just before, on the same (in-order) engine queue."""
    uid = 0
    for f in nc.m.functions:
        for bb in f.blocks:
            if not any(
                i.sync_info is not None and len(i.sync_info.on_wait) > 1
                for i in bb.instructions
            ):
                continue
            new_list = []
            for inst in bb.instructions:
                si = inst.sync_info
                if si is not None and len(si.on_wait) > 1:
                    waits = list(si.on_wait)
                    for w in waits[:-1]:
                        uid += 1
                        new_list.append(
                            mybir.InstNoOp(
                                name=f"WS-{uid}",
                                engine=inst.engine,
                                bass_nofuse=True,
                                sync_info=mybir.SyncInfo(on_wait=[w], on_update=[]),
                            )
                        )
                    inst.sync_info = mybir.SyncInfo(
                        on_wait=[waits[-1]], on_update=list(si.on_update)
                    )
                new_list.append(inst)
            bb.instructions = new_list


_NC_CACHE = None


def _build_nc():
    global _NC_CACHE
    if _NC_CACHE is not None:
        return _NC_CACHE
    nc = bass.Bass(trn_type="TRN2")
    # eye | wh | u-pair-0 packed bf16 so one DMA unblocks the whole scan start
    ew_d = nc.dram_tensor("ew", [G * H, 2 * G * H + 2 * BG], BF16, kind="ExternalInput")
    # pairs 1-3, DMA'd concurrently from the Scalar queue: 8 steps of headroom
    # before the first Sync-queue chunk DMA must land
    u03_d = nc.dram_tensor("u03", [G * H, 2 * (EWP - 1) * BG], BF16, kind="ExternalInput")
    u_d = nc.dram_tensor("u", [G * H, NF - 2 * EWP * BG], BF16, kind="ExternalInput")
    # [wd half0 (90) | wd half1 (90) | mask (171)] bf16 over 121 partitions;
    # row 120 = [bd tile | bd tile | ones] (bias contract row / hm ones row)
    wdb_d = nc.dram_tensor("wdb", [G * H + 1, 351], BF16, kind="ExternalInput")
    o_d = nc.dram_tensor("o", [90, 2 * BG], F32, kind="ExternalOutput")

    with TileContext(nc) as tc:
        with (
            tc.tile_pool(name="const", bufs=1) as cpool,
            tc.tile_pool(name="u", bufs=1) as upool,
            tc.tile_pool(name="work", bufs=4) as wpool,
            tc.tile_pool(name="fin", bufs=1) as fpool,
            tc.tile_pool(name="ps0", bufs=3, space="PSUM") as pp0,
            tc.tile_pool(name="ps1", bufs=3, space="PSUM") as pp1,
        ):
            ppools = [pp0, pp1]
            t_ew = cpool.tile([G * H, 2 * G * H + 2 * BG], BF16, tag="ew")
            nc.sync.dma_start(out=t_ew[:], in_=ew_d[:])
            t_eye = t_ew[:, 0:G * H]
            t_wh = t_ew[:, G * H:2 * G * H]
            t_u03 = cpool.tile([G * H, 2 * (EWP - 1) * BG], BF16, tag="u03")
            nc.scalar.dma_start(out=t_u03[:], in_=u03_d[:])

            # u chunk tiles; pair p (2 steps) occupies cols [342p, 342p+342)
            # grouped per chain: [c0(s),c0(s+1) | c1(s),c1(s+1) | ...]
            uts = []
            step0 = 2 * EWP
            for k, ns in enumerate(CHUNK_STEPS):
                ut = upool.tile([G * H, ns * BG], BF16, tag=f"u{k}")
                nc.sync.dma_start(
                    out=ut[:],
                    in_=u_d[:, (step0 - 2 * EWP) * BG:(step0 - 2 * EWP + ns) * BG],
                )
                uts.append((ut, step0))
                step0 += ns
            # pair p -> (tile, base column of its 342-col chain-grouped block)
            pair_src = {0: (t_ew, 2 * G * H)}
            for p in range(1, EWP):
                pair_src[p] = (t_u03, 2 * BG * (p - 1))
            for (ut, s0), ns in zip(uts, CHUNK_STEPS):
                for p in range(s0 // 2, (s0 + ns) // 2):
                    pair_src[p] = (ut, (p - s0 // 2) * 2 * BG)

            # tail constants (dense head) arrive long before they're needed
            t_wdb = cpool.tile([G * H + 1, 351], BF16, tag="wdb")
            nc.sync.dma_start(out=t_wdb[:], in_=wdb_d[:])
            t_mask = t_wdb[0:G * H, 180:351]

            # masked-h operand for the head: rows 0..119 = h_T * mask, row 120
            # = 1.0 (contracts against the bd row of the stationary). Engines
            # can't start a write at partition 120, so set the whole tile to
            # 1.0 and let the mask-muls overwrite rows 0..119.
            hm = fpool.tile([G * H + 1, BG], BF16, tag="hm")
            nc.vector.memset(hm[:], 1.0)

            hs = [None] * NCH
            for p in range(T // 2):
                ut, pbase = pair_src[p]
                banks = []
                for c in range(NCH):
                    w = CB[c + 1] - CB[c]
                    ps = ppools[c].tile([G * H, 2 * w], F32, tag=f"ps{c}")
                    off = pbase + 2 * CB[c]
                    if p == 0:
                        # h_{-1} = 0, so step 0 has no Wh term: close its eye
                        # injection immediately (start+stop) and let the first
                        # activation read it directly — the zero-h Wh matmul
                        # and the h0 memsets disappear from the startup path
                        nc.tensor.matmul(
                            ps[:, 0:w], t_eye, ut[:, off:off + w],
                            start=True, stop=True, skip_group_check=True,
                        )
                        nc.tensor.matmul(
                            ps[:, w:2 * w], t_eye, ut[:, off + w:off + 2 * w],
                            start=True, stop=False, skip_group_check=True,
                        )
                    else:
                        nc.tensor.matmul(
                            ps[:], t_eye, ut[:, off:off + 2 * w],
                            start=True, stop=False,
                        )
                    banks.append(ps)
                for s in range(2):
                    step = 2 * p + s
                    last = step == T - 1
                    for c in range(NCH):
                        w = CB[c + 1] - CB[c]
                        sl = banks[c][:, s * w:(s + 1) * w]
                        if step > 0:
                            nc.tensor.matmul(
                                sl, t_wh, hs[c][:],
                                start=False, stop=True, skip_group_check=True,
                            )
                        pool = fpool if last else wpool
                        h_new = pool.tile(
                            [G * H, w],
                            F32 if last else BF16,
                            tag=(f"fh{c}" if last else f"h{c}"),
                        )
                        nc.scalar.activation(
                            h_new[:], sl, mybir.ActivationFunctionType.Tanh
                        )
                        hs[c] = h_new

            for c in range(NCH):
                nc.vector.tensor_mul(
                    hm[0:G * H, CB[c]:CB[c + 1]], hs[c][:],
                    t_mask[:, CB[c]:CB[c + 1]],
                )
            ob = fpool.tile([90, 2 * BG], F32, tag="ob")
            for half in range(2):
                # reuse the chain tag: a new tag would cost another bufs x bank
                po = ppools[half].tile([90, BG], F32, tag=f"ps{half}")
                nc.tensor.matmul(
                    po[:], t_wdb[:, 90 * half:90 * (half + 1)], hm[:],
                    start=True, stop=True,
                )
                nc.vector.tensor_copy(ob[:, BG * half:BG * (half + 1)], po[:])
            nc.sync.dma_start(out=o_d[:], in_=ob[:])

    _split_multi_waits(nc)
    _NC_CACHE = nc
    return nc


def _prepare_in_maps(x, emb, Wx, Wh, b_rnn, Wd, bd, drop_mask):
    x = np.asarray(x)
    emb = np.asarray(emb, dtype=np.float32)
    Wx = np.asarray(Wx, dtype=np.float32)
    Wh = np.asarray(Wh, dtype=np.float32)
    b_rnn = np.asarray(b_rnn, dtype=np.float32)
    Wd = np.asarray(Wd, dtype=np.float32)
    bd = np.asarray(bd, dtype=np.float32)
    drop_mask = np.asarray(drop_mask, dtype=np.float32)

    M = emb @ Wx + b_rnn  # [V, H] fused embedding+input-proj table
    Mb = M.astype(NP_BF16)

    wh_blk = np.zeros((G * H, G * H), np.float32)
    wd_blk = np.zeros((G * H, 180), np.float32)
    for a in range(G):
        wh_blk[10 * a:10 * a + 10, 10 * a:10 * a + 10] = Wh
        half, b6 = divmod(a, 6)
        wd_blk[10 * a:10 * a + 10, 90 * half + 15 * b6:90 * half + 15 * b6 + 15] = Wd
    ew_base = np.concatenate(
        [np.eye(G * H, dtype=np.float32), wh_blk], axis=1
    ).astype(NP_BF16)

    in_maps = []
    for c in range(N_CORES):
        xs = x[c * BC:(c + 1) * BC].astype(np.int64)
        u = np.zeros((BP, T, H), NP_BF16)
        u[:BC] = Mb[xs]
        # [120, 17100]: u_dev[10g+h, 171t+j] = u[171g+j, t, h]
        u_dev = (
            u.reshape(G, BG, T, H).transpose(0, 3, 2, 1).reshape(G * H, NF)
        )
        # regroup columns per 2-step pair into chain blocks:
        # pair p -> [ch0(s), ch0(s+1), ch1(s), ch1(s+1), ...]
        v = u_dev.reshape(G * H, T // 2, 2, BG)
        u_dev = np.ascontiguousarray(
            np.concatenate(
                [
                    v[:, :, :, CB[ci]:CB[ci + 1]].reshape(
                        G * H, T // 2, 2 * (CB[ci + 1] - CB[ci])
                    )
                    for ci in range(NCH)
                ],
                axis=2,
            ).reshape(G * H, NF)
        )
        mp = np.zeros((BP, H), np.float32)
        mp[:BC] = drop_mask[c * BC:(c + 1) * BC]
        mask_dev = mp.reshape(G, BG, H).transpose(0, 2, 1).reshape(G * H, BG)
        wdb = np.zeros((G * H + 1, 351), np.float32)
        wdb[0:G * H, 0:180] = wd_blk
        wdb[0:G * H, 180:351] = mask_dev
        wdb[G * H, 0:90] = np.tile(bd, 6)
        wdb[G * H, 90:180] = np.tile(bd, 6)
        wdb[G * H, 180:351] = 1.0
        ew = np.ascontiguousarray(
            np.concatenate([ew_base, u_dev[:, 0:2 * BG]], axis=1)
        )
        u03 = np.ascontiguousarray(u_dev[:, 2 * BG:2 * EWP * BG])
        u_rest = np.ascontiguousarray(u_dev[:, 2 * EWP * BG:])
        in_maps.append(
            {"ew": ew, "u03": u03, "u": u_rest, "wdb": wdb.astype(NP_BF16)}
        )
    return in_maps


def _assemble(results):
    logits = np.empty((B, L), np.float32)
    for c in range(N_CORES):
        o = results[c]["o"]  # [90, 342]
        parts = []
        for half in range(2):
            oh = o[:, BG * half:BG * (half + 1)]  # [90, 171]
            parts.append(oh.reshape(6, 15, BG).transpose(0, 2, 1).reshape(6 * BG, 15))
        full = np.concatenate(parts, axis=0)  # [2052, 15]
        logits[c * BC:(c + 1) * BC] = full[:BC]
    return logits


_LAST_RES = None


def kernel(x, emb, Wx, Wh, b_rnn, Wd, bd, drop_mask, _trace=False):
    global _LAST_RES
    nc = _build_nc()
    in_maps = _prepare_in_maps(x, emb, Wx, Wh, b_rnn, Wd, bd, drop_mask)
    res = run_bass_kernel_spmd(
        nc, in_maps, core_ids=list(range(N_CORES)), trace=_trace
    )
    _LAST_RES = res
    out = _assemble(res.results)
    if _trace:
        kernel.last_exec_time_ns = res.exec_time_ns
    return out

